# revision 4
# baseline (speedup 1.0000x reference)
"""Causal self-attention (B=2, T=2048, D=2048, H=16, hd=128) on 8 trn2 cores.

Sharding: core c -> batch b=c//4, head group g=c%4 (heads 4g..4g+3).
Data parallel on B, tensor parallel on heads. Each core computes the
out-proj partial for its head group; host sums the 4 partials per batch.

Self-contained: hardcodes shapes; builds/compiles the Bass program once
per process and caches it.
"""

import sys

sys.path.insert(0, "/opt/trn_rl_repo")

from contextlib import ExitStack

import ml_dtypes
import numpy as np

import concourse.bass as bass
import concourse.tile as tile
from concourse import bacc, mybir
from concourse.bass_utils import run_bass_kernel_spmd
from concourse.masks import make_identity

BF16 = mybir.dt.bfloat16
F32 = mybir.dt.float32
NPBF16 = ml_dtypes.bfloat16

T = 2048
D = 2048
HD = 128
NH = 16
NHC = 4  # heads per core
OC = 3 * NHC * HD  # 1536 qkv out dims per core
SCALE = 1.0 / float(np.sqrt(HD))
N_CORES = 8


def build_nc(repeat: int = 1):
    """Emit the per-core SPMD program. repeat>1 wraps the body in a HW loop
    (used only for wall-clock delta timing)."""
    nc = bacc.Bacc("TRN2", target_bir_lowering=False, debug=False)

    xT_d = nc.dram_tensor("xT", [D, T], BF16, kind="ExternalInput").ap()
    wqkvT_d = nc.dram_tensor("wqkvT", [D, OC], BF16, kind="ExternalInput").ap()
    woutT_d = nc.dram_tensor("woutT", [NHC * HD, D], BF16, kind="ExternalInput").ap()
    cosT2_d = nc.dram_tensor("cosT2", [128, T], F32, kind="ExternalInput").ap()
    sinT2_d = nc.dram_tensor("sinT2", [128, T], F32, kind="ExternalInput").ap()
    mask_d = nc.dram_tensor("maskf", [128, 2048], BF16, kind="ExternalInput").ap()
    outp_d = nc.dram_tensor("outp", [T, D], F32, kind="ExternalOutput").ap()

    xTr = xT_d.rearrange("(i p) t -> i p t", p=128)  # [16,128,2048]
    wr = wqkvT_d.rearrange("(i p) o -> p i o", p=128)  # [128,16,1536]
    woutr = woutT_d.rearrange("(h p) j -> p h j", p=128)  # [128,4,2048]

    with tile.TileContext(nc) as tc, ExitStack() as ctx:
        const = ctx.enter_context(tc.tile_pool(name="const", bufs=1))
        ident = const.tile([128, 128], BF16, tag="ident")
        make_identity(nc, ident[:])
        ones = const.tile([128, 1], BF16, tag="ones")
        nc.gpsimd.memset(ones[:], 1.0)
        cos_sb = const.tile([128, T], F32, tag="cos")
        nc.sync.dma_start(cos_sb[:], cosT2_d[:])
        sin_sb = const.tile([128, T], F32, tag="sin")
        nc.sync.dma_start(sin_sb[:], sinT2_d[:])
        mask_sb = const.tile([128, 2048], BF16, tag="mask")
        nc.sync.dma_start(mask_sb[:], mask_d[:])
        wout_sb = const.tile([128, NHC, D], BF16, tag="wout")
        nc.sync.dma_start(wout_sb[:], woutr[:])

        def body(_it):
            with ExitStack() as bctx:
                qkv = bctx.enter_context(tc.tile_pool(name="qkv", bufs=1))
                qT = qkv.tile([128, NHC, T], BF16, tag="qT")
                kT = qkv.tile([128, NHC, T], BF16, tag="kT")
                vTt = qkv.tile([128, NHC, T], BF16, tag="vTt")
                v_sb = qkv.tile([128, 16, NHC * HD], BF16, tag="v")

                # ---- P1: qkv projection + rope --------------------------
                with ExitStack() as p1:
                    xp = p1.enter_context(tc.tile_pool(name="x", bufs=1))
                    xT_sb = xp.tile([128, 16, T], BF16, tag="xT")
                    for i in range(16):
                        nc.sync.dma_start(xT_sb[:, i], xTr[i])
                    wp = p1.enter_context(tc.tile_pool(name="w", bufs=2))
                    psp = p1.enter_context(
                        tc.tile_pool(name="projps", bufs=2, space="PSUM")
                    )
                    tmp = p1.enter_context(tc.tile_pool(name="ropetmp", bufs=2))
                    for ob in range(12):
                        wt = wp.tile([128, 16, 128], BF16, tag="w")
                        nc.sync.dma_start(wt[:], wr[:, :, ob * 128 : (ob + 1) * 128])
                        ps = psp.tile([128, T], F32, tag="projps")
                        for i in range(16):
                            for tcn in range(4):
                                nc.tensor.matmul(
                                    ps[:, tcn * 512 : (tcn + 1) * 512],
                                    wt[:, i],
                                    xT_sb[:, i, tcn * 512 : (tcn + 1) * 512],
                                    start=(i == 0),
                                    stop=(i == 15),
                                )
                        if ob < 8:
                            dst = qT if ob < 4 else kT
                            h = ob % 4
                            for tcn in range(4):
                                sl = slice(tcn * 512, (tcn + 1) * 512)
                                t1 = tmp.tile([64, 512], F32, tag="t1")
                                t2 = tmp.tile([64, 512], F32, tag="t2")
                                nc.vector.tensor_mul(t1[:], ps[0:64, sl], cos_sb[0:64, sl])
                                nc.vector.tensor_mul(t2[:], ps[64:128, sl], sin_sb[0:64, sl])
                                nc.vector.tensor_sub(dst[0:64, h, sl], t1[:], t2[:])
                                t3 = tmp.tile([64, 512], F32, tag="t1")
                                t4 = tmp.tile([64, 512], F32, tag="t2")
                                nc.vector.tensor_mul(t3[:], ps[64:128, sl], cos_sb[64:128, sl])
                                nc.vector.tensor_mul(t4[:], ps[0:64, sl], sin_sb[64:128, sl])
                                nc.vector.tensor_add(dst[64:128, h, sl], t3[:], t4[:])
                        else:
                            h = ob - 8
                            for tcn in range(4):
                                sl = slice(tcn * 512, (tcn + 1) * 512)
                                nc.scalar.activation(
                                    vTt[:, h, sl],
                                    ps[:, sl],
                                    mybir.ActivationFunctionType.Copy,
                                )

                # ---- P1b: V^T -> V via PE transpose ---------------------
                with tc.tile_pool(name="vtps", bufs=2, space="PSUM") as vtp:
                    for h in range(NHC):
                        for kb in range(16):
                            tp = vtp.tile([128, 128], BF16, tag="vtp")
                            nc.tensor.transpose(
                                tp[:], vTt[:, h, kb * 128 : (kb + 1) * 128], ident[:]
                            )
                            nc.vector.tensor_copy(
                                v_sb[:, kb, h * 128 : (h + 1) * 128], tp[:]
                            )

                # ---- P2: attention --------------------------------------
                oS = qkv.tile([128, NHC, T], BF16, tag="oS")
                with ExitStack() as p2:
                    stp = p2.enter_context(tc.tile_pool(name="stps", bufs=2, space="PSUM"))
                    avp = p2.enter_context(tc.tile_pool(name="avps", bufs=2, space="PSUM"))
                    rsp = p2.enter_context(tc.tile_pool(name="rsps", bufs=2, space="PSUM"))
                    ptp = p2.enter_context(tc.tile_pool(name="pt", bufs=3))
                    nrm = p2.enter_context(tc.tile_pool(name="norm", bufs=2))
                    for h in range(NHC):
                        for qc in range(4):
                            qsl = slice(qc * 512, (qc + 1) * 512)
                            oT = avp.tile([128, 512], F32, tag="av")
                            rs = rsp.tile([1, 512], F32, tag="rs")
                            nkb = 4 * qc + 4
                            pend = None  # software pipeline: AV/rs lag one kb
                            for kb in range(nkb):
                                st = stp.tile([128, 512], F32, tag="st")
                                nc.tensor.matmul(
                                    st[:],
                                    kT[:, h, kb * 128 : (kb + 1) * 128],
                                    qT[:, h, qsl],
                                    start=True,
                                    stop=True,
                                )
                                pt = ptp.tile([128, 512], BF16, tag="pt")
                                nc.scalar.activation(
                                    pt[:], st[:],
                                    mybir.ActivationFunctionType.Exp,
                                    scale=SCALE,
                                )
                                if kb >= 4 * qc:
                                    jj = kb - 4 * qc
                                    nc.vector.tensor_mul(
                                        pt[:], pt[:],
                                        mask_sb[:, jj * 512 : (jj + 1) * 512],
                                    )
                                if pend is not None:
                                    pkb, ppt = pend
                                    nc.tensor.matmul(
                                        oT[:],
                                        v_sb[:, pkb, h * 128 : (h + 1) * 128],
                                        ppt[:],
                                        start=(pkb == 0), stop=False,
                                    )
                                    nc.tensor.matmul(
                                        rs[:], ones[:], ppt[:],
                                        start=(pkb == 0), stop=False,
                                    )
                                pend = (kb, pt)
                            pkb, ppt = pend
                            nc.tensor.matmul(
                                oT[:], v_sb[:, pkb, h * 128 : (h + 1) * 128], ppt[:],
                                start=(pkb == 0), stop=True,
                            )
                            nc.tensor.matmul(
                                rs[:], ones[:], ppt[:],
                                start=(pkb == 0), stop=True,
                            )
                            recip = nrm.tile([1, 512], F32, tag="recip")
                            nc.vector.reciprocal(recip[:], rs[:])
                            rb = nrm.tile([128, 512], F32, tag="rb")
                            nc.gpsimd.partition_broadcast(rb[:], recip[:])
                            nc.vector.tensor_mul(oS[:, h, qsl], oT[:], rb[:])

                # ---- P3: out projection ---------------------------------
                with tc.tile_pool(name="opps", bufs=2, space="PSUM") as opp, \
                        tc.tile_pool(name="ostage", bufs=3) as osg:
                    for qt in range(16):
                        ops = opp.tile([128, D], F32, tag="op")
                        for dh in range(NHC):
                            for jc in range(4):
                                nc.tensor.matmul(
                                    ops[:, jc * 512 : (jc + 1) * 512],
                                    oS[:, dh, qt * 128 : (qt + 1) * 128],
                                    wout_sb[:, dh, jc * 512 : (jc + 1) * 512],
                                    start=(dh == 0),
                                    stop=(dh == NHC - 1),
                                )
                        stg = osg.tile([128, D], F32, tag="stg")
                        for jc in range(4):
                            jsl = slice(jc * 512, (jc + 1) * 512)
                            eng = nc.scalar if jc % 2 == 0 else nc.vector
                            if jc % 2 == 0:
                                nc.scalar.activation(
                                    stg[:, jsl], ops[:, jsl],
                                    mybir.ActivationFunctionType.Copy,
                                )
                            else:
                                nc.vector.tensor_copy(stg[:, jsl], ops[:, jsl])
                        nc.sync.dma_start(outp_d[qt * 128 : (qt + 1) * 128, :], stg[:])

        if repeat == 1:
            body(0)
        else:
            with tc.For_i(0, repeat, 1) as it:
                body(it)

    nc.compile()
    return nc


def make_in_maps(x, cos, sin, W_qkv, W_out):
    """Host-side sharding/layout prep. Pure layout: transpose + slice + cast."""
    cosT2 = np.vstack([cos[0, :, :64].T, cos[0, :, :64].T]).astype(np.float32)
    sinT2 = np.vstack([sin[0, :, :64].T, sin[0, :, :64].T]).astype(np.float32)
    kk = np.arange(128)[:, None]
    cc = np.arange(512)[None, :]
    maskf = np.zeros((128, 2048), dtype=NPBF16)
    for jj in range(4):
        maskf[:, jj * 512 : (jj + 1) * 512] = (cc >= 128 * jj + kk).astype(NPBF16)

    in_maps = []
    for c in range(N_CORES):
        b, g = c // 4, c % 4
        r = slice(512 * g, 512 * (g + 1))
        wq = W_qkv[0:2048][r]
        wk = W_qkv[2048:4096][r]
        wv = W_qkv[4096:6144][r]
        wqkvT = np.ascontiguousarray(
            np.concatenate([wq, wk, wv], axis=0).T
        ).astype(NPBF16)
        woutT = np.ascontiguousarray(W_out[:, r].T).astype(NPBF16)
        xT = np.ascontiguousarray(x[b].T).astype(NPBF16)
        in_maps.append(
            dict(xT=xT, wqkvT=wqkvT, woutT=woutT, cosT2=cosT2, sinT2=sinT2,
                 maskf=maskf)
        )
    return in_maps


_NC_CACHE = {}


def get_nc(repeat: int = 1):
    if repeat not in _NC_CACHE:
        _NC_CACHE[repeat] = build_nc(repeat)
    return _NC_CACHE[repeat]


def kernel(x, cos, sin, W_qkv, W_out):
    x = np.asarray(x, dtype=np.float32)
    cos = np.asarray(cos, dtype=np.float32)
    sin = np.asarray(sin, dtype=np.float32)
    W_qkv = np.asarray(W_qkv, dtype=np.float32)
    W_out = np.asarray(W_out, dtype=np.float32)

    nc = get_nc(1)
    in_maps = make_in_maps(x, cos, sin, W_qkv, W_out)
    res = run_bass_kernel_spmd(nc, in_maps, list(range(N_CORES)))
    out = np.zeros((2, T, D), dtype=np.float32)
    for c in range(N_CORES):
        out[c // 4] += res.results[c]["outp"]
    return out
